# revision 1
# baseline (speedup 1.0000x reference)
"""nn_Block_15857019256918: windowed-attention transformer block on 8 trn2 cores.

Sharding: data-parallel over the B*25=100 attention windows (padded to 104 so
each of the 8 cores owns 13 windows). Every real token (b,h,w) belongs to
exactly one 14x14 window, so the residual + MLP for those tokens stays on the
same core — no cross-core communication at all. Weights are replicated.

Host does layout-only work (pad, window partition, static rel-pos gather,
unpartition); all FLOPs run on the 8 NeuronCores via one pmapped program.
"""

import numpy as np
import jax
import jax.numpy as jnp

DIM = 768
NH = 12
HD = DIM // NH
WS = 14
EPS = 1e-5
B, H, W = 4, 64, 64
NWIN_SIDE = 5           # ceil(64/14)
NWIN = B * NWIN_SIDE * NWIN_SIDE   # 100
NCORES = 8
NWIN_PAD = 104          # 8 * 13
N = WS * WS             # 196


def _ln(x, w, b):
    m = jnp.mean(x, -1, keepdims=True)
    v = jnp.var(x, -1, keepdims=True)
    return (x - m) * jax.lax.rsqrt(v + EPS) * w + b


def _core_fn(xw, mask, ln1_w, ln1_b, qkv_w, qkv_b, proj_w, proj_b,
             Rh, Rw, ln2_w, ln2_b, fc1_w, fc1_b, fc2_w, fc2_b):
    # xw: [nw, N, DIM] raw tokens (zero in pad region); mask: [nw, N, 1]
    # Heavy GEMMs run with bf16 operands + f32 accumulation (2x TensorE rate);
    # LN, softmax, gelu, residuals stay f32.
    bf = jnp.bfloat16
    f32 = jnp.float32
    nw = xw.shape[0]
    xn = _ln(xw, ln1_w, ln1_b) * mask          # pad rows forced to 0, as in ref

    qkv = jnp.matmul(xn.astype(bf), qkv_w.astype(bf),
                     preferred_element_type=f32) + qkv_b
    qkv = qkv.reshape(nw, N, 3, NH, HD).transpose(2, 0, 3, 1, 4)
    q, k, v = qkv[0], qkv[1], qkv[2]            # [nw, NH, N, HD] f32
    scale = HD ** -0.5
    attn = jnp.einsum("bhnd,bhmd->bhnm", (q * scale).astype(bf), k.astype(bf),
                      preferred_element_type=f32)

    rq = q.reshape(nw, NH, WS, WS, HD)
    rel_h = jnp.einsum("bnhwc,hkc->bnhwk", rq, Rh)
    rel_w = jnp.einsum("bnhwc,wkc->bnhwk", rq, Rw)
    attn = (attn.reshape(nw, NH, WS, WS, WS, WS)
            + rel_h[..., :, None] + rel_w[..., None, :]).reshape(nw, NH, N, N)

    attn = jax.nn.softmax(attn, axis=-1)
    out = jnp.einsum("bhnm,bhmd->bhnd", attn.astype(bf), v.astype(bf),
                     preferred_element_type=f32)
    out = out.transpose(0, 2, 1, 3).reshape(nw, N, DIM)
    out = jnp.matmul(out.astype(bf), proj_w.astype(bf),
                     preferred_element_type=f32) + proj_b

    tok = xw + out                              # residual (pad rows are garbage, dropped later)

    h = _ln(tok, ln2_w, ln2_b)
    h = jax.nn.gelu(jnp.matmul(h.astype(bf), fc1_w.astype(bf),
                               preferred_element_type=f32) + fc1_b,
                    approximate=False)
    return tok + (jnp.matmul(h.astype(bf), fc2_w.astype(bf),
                             preferred_element_type=f32) + fc2_b)


_pmapped = None


def _get_pmapped():
    global _pmapped
    if _pmapped is None:
        _pmapped = jax.pmap(
            _core_fn,
            in_axes=(0, 0) + (None,) * 14,
            devices=jax.devices()[:NCORES],
        )
    return _pmapped


def kernel(x, ln1_w, ln1_b, qkv_w, qkv_b, proj_w, proj_b,
           rel_pos_h, rel_pos_w, ln2_w, ln2_b, fc1_w, fc1_b, fc2_w, fc2_b):
    x = np.asarray(x, np.float32)

    # ---- host: window partition (layout only) ----
    xp = np.zeros((B, 70, 70, DIM), np.float32)
    xp[:, :H, :W, :] = x
    xw = xp.reshape(B, NWIN_SIDE, WS, NWIN_SIDE, WS, DIM).transpose(0, 1, 3, 2, 4, 5)
    xw = xw.reshape(NWIN, N, DIM)
    xw_pad = np.zeros((NWIN_PAD, N, DIM), np.float32)
    xw_pad[:NWIN] = xw
    xw_sh = xw_pad.reshape(NCORES, NWIN_PAD // NCORES, N, DIM)

    # per-window-position validity mask (1=real token, 0=pad)
    hreal = np.minimum(WS, H - WS * np.arange(NWIN_SIDE))        # [5]
    rowm = (np.arange(WS)[None, :] < hreal[:, None]).astype(np.float32)  # [5,14]
    m2 = np.einsum("ri,cj->rcij", rowm, rowm).reshape(NWIN_SIDE, NWIN_SIDE, N, 1)
    mask = np.broadcast_to(m2[None], (B, NWIN_SIDE, NWIN_SIDE, N, 1)).reshape(NWIN, N, 1)
    mask_pad = np.zeros((NWIN_PAD, N, 1), np.float32)
    mask_pad[:NWIN] = mask
    mask_sh = mask_pad.reshape(NCORES, NWIN_PAD // NCORES, N, 1)

    # static relative-position gather on host (indices depend only on shapes)
    idx = np.arange(WS)[:, None] - np.arange(WS)[None, :] + (WS - 1)
    Rh = np.asarray(rel_pos_h, np.float32)[idx]   # [WS, WS, HD]
    Rw = np.asarray(rel_pos_w, np.float32)[idx]

    out_sh = _get_pmapped()(
        xw_sh, mask_sh,
        jnp.asarray(ln1_w), jnp.asarray(ln1_b),
        jnp.asarray(qkv_w), jnp.asarray(qkv_b),
        jnp.asarray(proj_w), jnp.asarray(proj_b),
        jnp.asarray(Rh), jnp.asarray(Rw),
        jnp.asarray(ln2_w), jnp.asarray(ln2_b),
        jnp.asarray(fc1_w), jnp.asarray(fc1_b),
        jnp.asarray(fc2_w), jnp.asarray(fc2_b),
    )
    out = np.asarray(out_sh, np.float32).reshape(NWIN_PAD, N, DIM)[:NWIN]

    # ---- host: window unpartition + crop ----
    out = out.reshape(B, NWIN_SIDE, NWIN_SIDE, WS, WS, DIM).transpose(0, 1, 3, 2, 4, 5)
    out = out.reshape(B, 70, 70, DIM)[:, :H, :W, :]
    return np.ascontiguousarray(out, np.float32)

